# revision 4
# baseline (speedup 1.0000x reference)
"""Grouped top-1 sparse autoencoder (vq_codebook) on 8 Trainium2 NeuronCores.

Sharding: expert-style over the K=32 group axis (4 groups per core) for the
encode GEMM + top-1 + gather-decode; the per-core partial reconstructions are
ReduceScatter-summed on device so each core finishes the FVU partial sums for
its 256-token slice. Host work: input layout prep (transpose/slice), and
summing the tiny per-core partial outputs (e, sum(xc^2), column sums,
feature counts).

kernel(**inputs) takes the full unsharded inputs and returns
(fvu, feature_counts, aux_fvu) matching reference().
"""

import sys

for _p in ("/opt/trn_rl_repo",):
    if _p not in sys.path:
        sys.path.insert(0, _p)

import numpy as np
import concourse.bass as bass
import concourse.bacc as bacc
import concourse.tile as tile
from concourse import mybir
from concourse.bass_utils import run_bass_kernel_spmd

dt = mybir.dt
Alu = mybir.AluOpType

P = 128
K = 32  # groups
G = 2048  # latents per group
D = 2048  # d_model
B = 2048  # batch
NCORE = 8
KL = K // NCORE  # 4 groups per core
BS = B // NCORE  # 256 tokens per core for the fvu phase
NDT = D // P  # 16 contraction tiles
CH = 256  # encode matmul moving chunk (N)
NCH = G // CH  # 8 chunks per group
HALF = B // 2  # 1024 tokens per resident half
BT_H = HALF // P  # 8 B-tiles per half
NBT = B // P  # 16 B-tiles total

_nc_cache = None


def _build():
    nc = bacc.Bacc("TRN2", target_bir_lowering=False, debug=False, num_devices=NCORE)

    xT = nc.dram_tensor("xT", [D, B], dt.float32, kind="ExternalInput").ap()
    xs = nc.dram_tensor("xs", [BS, D], dt.float32, kind="ExternalInput").ap()
    b_decT = nc.dram_tensor("b_decT", [P, NDT], dt.float32, kind="ExternalInput").ap()
    b_dec_row = nc.dram_tensor("b_dec_row", [1, D], dt.float32, kind="ExternalInput").ap()
    encT = nc.dram_tensor("encT", [KL, D, G], dt.float32, kind="ExternalInput").ap()
    b_enc = nc.dram_tensor("b_enc", [KL, G], dt.float32, kind="ExternalInput").ap()
    dec = nc.dram_tensor("dec", [KL * G, D], dt.float32, kind="ExternalInput").ap()

    counts_part = nc.dram_tensor("counts_part", [KL, G], dt.int32, kind="ExternalOutput").ap()
    e_part = nc.dram_tensor("e_part", [1, 1], dt.float32, kind="ExternalOutput").ap()
    sq_part = nc.dram_tensor("sq_part", [1, 1], dt.float32, kind="ExternalOutput").ap()
    colsum_part = nc.dram_tensor("colsum_part", [1, D], dt.float32, kind="ExternalOutput").ap()
    idx_dbg = nc.dram_tensor("idx_dbg", [B, KL], dt.uint32, kind="ExternalOutput").ap()
    maxv_dbg = nc.dram_tensor("maxv_dbg", [B, KL], dt.float32, kind="ExternalOutput").ap()

    out_part = nc.dram_tensor("out_part", [B, D], dt.float32).ap()
    rs_out = nc.dram_tensor("rs_out", [BS, D], dt.float32).ap()

    with tile.TileContext(nc) as tc:
        with tc.tile_pool(name="const", bufs=1) as cpool:
            b_decT_sb = cpool.tile([P, NDT], dt.float32)
            nc.sync.dma_start(b_decT_sb[:], b_decT)
            iota_f = cpool.tile([P, G], dt.float32)
            iota_i = cpool.tile([P, G], dt.int32)
            nc.gpsimd.iota(iota_i[:], pattern=[[1, G]], base=0, channel_multiplier=0)
            nc.vector.tensor_copy(iota_f[:], iota_i[:])
            ones_bf = cpool.tile([P, 1], dt.bfloat16)
            nc.any.memset(ones_bf[:], 1.0)
            ones_f = cpool.tile([P, 1], dt.float32)
            nc.any.memset(ones_f[:], 1.0)
            idx_all = cpool.tile([P, NBT, KL], dt.uint32)
            maxv_all = cpool.tile([P, NBT, KL], dt.float32)

            # ================= encode =================
            with tc.tile_pool(name="enc_xc", bufs=1) as xc_pool, tc.tile_pool(
                name="enc_w", bufs=2
            ) as w_pool, tc.tile_pool(name="enc_benc", bufs=1) as benc_pool, tc.tile_pool(
                name="enc_acts", bufs=BT_H
            ) as acts_pool, tc.tile_pool(
                name="enc_small", bufs=4
            ) as small_pool, tc.tile_pool(
                name="enc_psum", bufs=4, space="PSUM"
            ) as mm_psum:
                for half in range(2):
                    xcT = xc_pool.tile([P, NDT, HALF], dt.float32, tag="xcT")
                    nc.sync.dma_start(
                        xcT[:],
                        xT.rearrange("(t p) b -> p t b", p=P)[
                            :, :, half * HALF : (half + 1) * HALF
                        ],
                    )
                    for t in range(NDT):
                        nc.vector.tensor_scalar(
                            out=xcT[:, t, :], in0=xcT[:, t, :],
                            scalar1=b_decT_sb[:, t : t + 1], scalar2=None,
                            op0=Alu.subtract,
                        )
                    for k in range(KL):
                        benc_rep = benc_pool.tile([P, G], dt.float32, tag="benc")
                        nc.sync.dma_start(
                            benc_rep[:], b_enc[k : k + 1, :].to_broadcast((P, G))
                        )
                        acts_tiles = [
                            acts_pool.tile([P, G], dt.float32, tag="acts", name="acts")
                            for _ in range(BT_H)
                        ]
                        for c in range(NCH):
                            w = w_pool.tile([P, NDT, CH], dt.float32, tag="w")
                            nc.sync.dma_start(
                                w[:],
                                encT[k].rearrange("(t p) g -> p t g", p=P)[
                                    :, :, c * CH : (c + 1) * CH
                                ],
                            )
                            for bt in range(BT_H):
                                ps = mm_psum.tile([P, CH], dt.float32, tag="mm")
                                for t in range(NDT):
                                    nc.tensor.matmul(
                                        ps[:],
                                        xcT[:, t, bt * P : (bt + 1) * P],
                                        w[:, t, :],
                                        start=(t == 0),
                                        stop=(t == NDT - 1),
                                    )
                                nc.vector.tensor_tensor(
                                    out=acts_tiles[bt][:, c * CH : (c + 1) * CH],
                                    in0=ps[:],
                                    in1=benc_rep[:, c * CH : (c + 1) * CH],
                                    op=Alu.add,
                                )
                        for bt in range(BT_H):
                            gbt = half * BT_H + bt
                            mx8 = small_pool.tile([P, 8], dt.float32, tag="mx8")
                            nc.vector.max(mx8[:], acts_tiles[bt][:])
                            mi8 = small_pool.tile([P, 8], dt.uint32, tag="mi8")
                            nc.vector.max_index(mi8[:], mx8[:], acts_tiles[bt][:])
                            nc.vector.tensor_copy(
                                maxv_all[:, gbt, k : k + 1], mx8[:, 0:1]
                            )
                            nc.vector.tensor_copy(
                                idx_all[:, gbt, k : k + 1], mi8[:, 0:1]
                            )

            nc.sync.dma_start(idx_dbg.rearrange("(t p) k -> p t k", p=P), idx_all[:])
            nc.sync.dma_start(maxv_dbg.rearrange("(t p) k -> p t k", p=P), maxv_all[:])

            # ================= counts + decode (overlap) =================
            with tc.tile_pool(name="pd_gath", bufs=3) as gath_pool, tc.tile_pool(
                name="pd_acc", bufs=2
            ) as acc_pool, tc.tile_pool(name="pd_small", bufs=6) as psmall, tc.tile_pool(
                name="pd_cnt_psum", bufs=1, space="PSUM"
            ) as cnt_psum, tc.tile_pool(name="pd_cnt_sb", bufs=4) as cnt_sb_pool:
                # ---- feature counts per local group
                for k in range(KL):
                    cps = [cnt_psum.tile([1, 512], dt.float32, tag=f"cnt{c}", name=f"cnt{c}") for c in range(4)]
                    for bt in range(NBT):
                        idx_f = psmall.tile([P, 1], dt.float32, tag="idxf")
                        nc.vector.tensor_copy(idx_f[:], idx_all[:, bt, k : k + 1])
                        mask = psmall.tile([P, G], dt.bfloat16, tag="mask")
                        nc.vector.tensor_scalar(
                            out=mask[:], in0=iota_f[:], scalar1=idx_f[:],
                            scalar2=None, op0=Alu.is_equal,
                        )
                        for c in range(4):
                            nc.tensor.matmul(
                                cps[c][:],
                                ones_bf[:],
                                mask[:, c * 512 : (c + 1) * 512],
                                start=(bt == 0),
                                stop=(bt == NBT - 1),
                            )
                    for c in range(4):
                        cnt_i = cnt_sb_pool.tile([1, 512], dt.int32, tag="cnti")
                        nc.vector.tensor_copy(cnt_i[:], cps[c][:])
                        nc.sync.dma_start(
                            counts_part[k : k + 1, c * 512 : (c + 1) * 512], cnt_i[:]
                        )

                # ---- decode: out_part[bt] = sum_k maxv * dec[k*G + idx]
                for bt in range(NBT):
                    acc = acc_pool.tile([P, D], dt.float32, tag="acc")
                    for k in range(KL):
                        idxg = psmall.tile([P, 1], dt.uint32, tag="idxg")
                        nc.vector.tensor_scalar(
                            out=idxg[:], in0=idx_all[:, bt, k : k + 1],
                            scalar1=k * G, scalar2=None, op0=Alu.add,
                        )
                        gath = gath_pool.tile([P, D], dt.float32, tag="gath")
                        nc.gpsimd.indirect_dma_start(
                            out=gath[:], out_offset=None, in_=dec,
                            in_offset=bass.IndirectOffsetOnAxis(ap=idxg[:], axis=0),
                        )
                        mv = maxv_all[:, bt, k : k + 1]
                        if k == 0:
                            nc.vector.tensor_scalar(
                                out=acc[:], in0=gath[:], scalar1=mv,
                                scalar2=None, op0=Alu.mult,
                            )
                        else:
                            nc.vector.scalar_tensor_tensor(
                                out=acc[:], in0=gath[:], scalar=mv, in1=acc[:],
                                op0=Alu.mult, op1=Alu.add,
                            )
                    nc.sync.dma_start(out_part[bt * P : (bt + 1) * P, :], acc[:])

            # ================= reduce-scatter =================
            nc.gpsimd.collective_compute(
                "ReduceScatter",
                Alu.add,
                replica_groups=[list(range(NCORE))],
                ins=[out_part],
                outs=[rs_out],
            )

            # ================= fvu partials on own 256-token slice =================
            with tc.tile_pool(name="fv", bufs=2) as fv_pool, tc.tile_pool(
                name="fv_acc", bufs=1
            ) as fvacc_pool, tc.tile_pool(name="fv_psum", bufs=1, space="PSUM") as fv_psum:
                bdrep = fvacc_pool.tile([P, D], dt.float32)
                nc.sync.dma_start(bdrep[:], b_dec_row.to_broadcast((P, D)))
                e_acc = fvacc_pool.tile([P, 1], dt.float32)
                nc.any.memset(e_acc[:], 0.0)
                sq_acc = fvacc_pool.tile([P, 1], dt.float32)
                nc.any.memset(sq_acc[:], 0.0)
                csps = [fv_psum.tile([1, 512], dt.float32, tag=f"cs{c}", name=f"cs{c}") for c in range(4)]
                nbt_s = BS // P  # 2
                for bt in range(nbt_s):
                    xc = fv_pool.tile([P, D], dt.float32, tag="xc")
                    nc.sync.dma_start(xc[:], xs[bt * P : (bt + 1) * P, :])
                    nc.vector.tensor_tensor(out=xc[:], in0=xc[:], in1=bdrep[:], op=Alu.subtract)
                    ot = fv_pool.tile([P, D], dt.float32, tag="ot")
                    nc.sync.dma_start(ot[:], rs_out[bt * P : (bt + 1) * P, :])
                    # diff = xc - (ot + b_dec)
                    nc.vector.tensor_tensor(out=ot[:], in0=ot[:], in1=bdrep[:], op=Alu.add)
                    nc.vector.tensor_tensor(out=ot[:], in0=xc[:], in1=ot[:], op=Alu.subtract)
                    nc.vector.tensor_tensor(out=ot[:], in0=ot[:], in1=ot[:], op=Alu.mult)
                    red = fv_pool.tile([P, 1], dt.float32, tag="red")
                    nc.vector.tensor_reduce(red[:], ot[:], axis=mybir.AxisListType.X, op=Alu.add)
                    nc.vector.tensor_tensor(out=e_acc[:], in0=e_acc[:], in1=red[:], op=Alu.add)
                    for c in range(4):
                        nc.tensor.matmul(
                            csps[c][:], ones_f[:], xc[:, c * 512 : (c + 1) * 512],
                            start=(bt == 0), stop=(bt == nbt_s - 1),
                        )
                    nc.vector.tensor_tensor(out=xc[:], in0=xc[:], in1=xc[:], op=Alu.mult)
                    red2 = fv_pool.tile([P, 1], dt.float32, tag="red2")
                    nc.vector.tensor_reduce(red2[:], xc[:], axis=mybir.AxisListType.X, op=Alu.add)
                    nc.vector.tensor_tensor(out=sq_acc[:], in0=sq_acc[:], in1=red2[:], op=Alu.add)
                for c in range(4):
                    cs_sb = fv_pool.tile([1, 512], dt.float32, tag="cs_sb")
                    nc.vector.tensor_copy(cs_sb[:], csps[c][:])
                    nc.sync.dma_start(colsum_part[:, c * 512 : (c + 1) * 512], cs_sb[:])
                e_s = fv_pool.tile([1, 1], dt.float32, tag="es")
                nc.gpsimd.tensor_reduce(e_s[:], e_acc[:], axis=mybir.AxisListType.XYZWC, op=Alu.add)
                nc.sync.dma_start(e_part, e_s[:])
                sq_s = fv_pool.tile([1, 1], dt.float32, tag="sqs")
                nc.gpsimd.tensor_reduce(sq_s[:], sq_acc[:], axis=mybir.AxisListType.XYZWC, op=Alu.add)
                nc.sync.dma_start(sq_part, sq_s[:])

    nc.compile()
    return nc


def _get_nc():
    global _nc_cache
    if _nc_cache is None:
        _nc_cache = _build()
    return _nc_cache


def kernel(x, encoder, b_enc, decoder, b_dec, _trace=False):
    x = np.ascontiguousarray(x, dtype=np.float32)
    encoder = np.asarray(encoder, dtype=np.float32)
    decoder = np.asarray(decoder, dtype=np.float32)
    b_enc = np.ascontiguousarray(b_enc, dtype=np.float32)
    b_dec = np.ascontiguousarray(b_dec, dtype=np.float32)

    xT = np.ascontiguousarray(x.T)
    b_decT = np.ascontiguousarray(b_dec.reshape(NDT, P).T)
    b_dec_row = np.ascontiguousarray(b_dec.reshape(1, D))

    in_maps = []
    for c in range(NCORE):
        ks = slice(c * KL, (c + 1) * KL)
        in_maps.append(
            dict(
                xT=xT,
                xs=np.ascontiguousarray(x[c * BS : (c + 1) * BS]),
                b_decT=b_decT,
                b_dec_row=b_dec_row,
                encT=np.ascontiguousarray(encoder[ks].transpose(0, 2, 1)),
                b_enc=np.ascontiguousarray(b_enc[ks]),
                dec=np.ascontiguousarray(decoder[ks].reshape(KL * G, D)),
            )
        )

    nc = _get_nc()
    res = run_bass_kernel_spmd(nc, in_maps, list(range(NCORE)), trace=_trace)
    results = res.results

    counts = np.concatenate(
        [results[c]["counts_part"].reshape(-1) for c in range(NCORE)]
    ).astype(np.int32)
    e = np.float64(sum(float(results[c]["e_part"][0, 0]) for c in range(NCORE)))
    sq = np.float64(sum(float(results[c]["sq_part"][0, 0]) for c in range(NCORE)))
    colsum = np.sum(
        [results[c]["colsum_part"][0].astype(np.float64) for c in range(NCORE)], axis=0
    )
    tv = sq - float(np.sum(colsum * colsum)) / B
    fvu = np.float32(e / tv)
    if _trace:
        kernel._last_perf = res
    return fvu, counts, np.float32(0.0)


if __name__ == "__main__":
    # smoke build
    _get_nc()
    print("build+compile OK")


# revision 5
# speedup vs baseline: 1.2152x; 1.2152x over previous
"""Grouped top-1 sparse autoencoder (vq_codebook) on 8 Trainium2 NeuronCores.

Sharding: expert-style over the K=32 group axis (4 groups per core) for the
encode GEMM + top-1 + gather-decode; per-core partial reconstructions are
ReduceScatter-summed on device (split in two halves to overlap with decode)
so each core finishes the FVU partial sums for its 256-token slice.

Encode GEMM runs in float32r (TRN2 tfloat32: 12-bit significand) with an
exact hi/lo split of both operands done on the host; three f32r passes
(hi*hi + hi*lo + lo*hi) reproduce fp32 accuracy at ~1 cycle/row instead of
fp32's 4. b_dec is folded into an effective encoder bias computed in fp64 on
the host: (x-b_dec)@enc^T + b_enc == x@enc^T + (b_enc - enc@b_dec).

Host work: input layout prep (transpose/slice/split) and summing the tiny
per-core partials (e, sum(xc^2), column sums, feature counts).
"""

import sys

for _p in ("/opt/trn_rl_repo",):
    if _p not in sys.path:
        sys.path.insert(0, _p)

import numpy as np
import concourse.bass as bass
import concourse.bacc as bacc
import concourse.tile as tile
from concourse import mybir
from concourse.bass_utils import run_bass_kernel_spmd

dt = mybir.dt
Alu = mybir.AluOpType

P = 128
K = 32  # groups
G = 2048  # latents per group
D = 2048  # d_model
B = 2048  # batch
NCORE = 8
KL = K // NCORE  # 4 groups per core
BS = B // NCORE  # 256 tokens per core for the fvu phase
NDT = D // P  # 16 contraction tiles
CH = 512  # encode matmul moving chunk (N)
NCH = G // CH  # 4 chunks per group
QB = B // 4  # 512 tokens per resident x block
BT_Q = QB // P  # 4 B-tiles per block
NBT = B // P  # 16 B-tiles total

_nc_cache = None


def _f32r_round(x: np.ndarray) -> np.ndarray:
    """Round fp32 to nearest 12-bit-significand value (matches PE f32r RNE)."""
    u = x.astype(np.float32).view(np.uint32).astype(np.uint64)
    drop = 12
    bias = ((u >> np.uint64(drop)) & np.uint64(1)) + (
        (np.uint64(1) << np.uint64(drop - 1)) - np.uint64(1)
    )
    u2 = ((u + bias) & ~np.uint64((1 << drop) - 1)).astype(np.uint32)
    return u2.view(np.float32)


def _build():
    nc = bacc.Bacc("TRN2", target_bir_lowering=False, debug=False, num_devices=NCORE)

    xTh = nc.dram_tensor("xTh", [D, B], dt.float32r, kind="ExternalInput").ap()
    xTl = nc.dram_tensor("xTl", [D, B], dt.float32r, kind="ExternalInput").ap()
    xs = nc.dram_tensor("xs", [BS, D], dt.float32, kind="ExternalInput").ap()
    b_dec_row = nc.dram_tensor("b_dec_row", [1, D], dt.float32, kind="ExternalInput").ap()
    encTh = nc.dram_tensor("encTh", [KL, D, G], dt.float32r, kind="ExternalInput").ap()
    encTl = nc.dram_tensor("encTl", [KL, D, G], dt.float32r, kind="ExternalInput").ap()
    benc_eff = nc.dram_tensor("benc_eff", [KL, G], dt.float32, kind="ExternalInput").ap()
    dec = nc.dram_tensor("dec", [KL * G, D], dt.float32, kind="ExternalInput").ap()

    counts_part = nc.dram_tensor("counts_part", [KL, G], dt.int32, kind="ExternalOutput").ap()
    e_part = nc.dram_tensor("e_part", [1, 1], dt.float32, kind="ExternalOutput").ap()
    sq_part = nc.dram_tensor("sq_part", [1, 1], dt.float32, kind="ExternalOutput").ap()
    colsum_part = nc.dram_tensor("colsum_part", [1, D], dt.float32, kind="ExternalOutput").ap()
    idx_dbg = nc.dram_tensor("idx_dbg", [B, KL], dt.uint32, kind="ExternalOutput").ap()
    maxv_dbg = nc.dram_tensor("maxv_dbg", [B, KL], dt.float32, kind="ExternalOutput").ap()

    # partial reconstructions, split so RS(A) can overlap decode of odd tiles.
    # out_partA rows c*128+r hold token 256c+r   (even b-tiles 2c)
    # out_partB rows c*128+r hold token 256c+128+r (odd b-tiles 2c+1)
    out_partA = nc.dram_tensor("out_partA", [B // 2, D], dt.float32).ap()
    out_partB = nc.dram_tensor("out_partB", [B // 2, D], dt.float32).ap()
    rs_outA = nc.dram_tensor("rs_outA", [P, D], dt.float32).ap()
    rs_outB = nc.dram_tensor("rs_outB", [P, D], dt.float32).ap()

    with tile.TileContext(nc) as tc:
        with tc.tile_pool(name="const", bufs=1) as cpool:
            iota_f = cpool.tile([P, G], dt.float32)
            iota_i = cpool.tile([P, G], dt.int32)
            nc.gpsimd.iota(iota_i[:], pattern=[[1, G]], base=0, channel_multiplier=0)
            nc.vector.tensor_copy(iota_f[:], iota_i[:])
            ones_bf = cpool.tile([P, 1], dt.bfloat16)
            nc.any.memset(ones_bf[:], 1.0)
            ones_f = cpool.tile([P, 1], dt.float32)
            nc.any.memset(ones_f[:], 1.0)
            idx_all = cpool.tile([P, NBT, KL], dt.uint32)
            maxv_all = cpool.tile([P, NBT, KL], dt.float32)

            # ================= encode =================
            with tc.tile_pool(name="enc_x", bufs=1) as x_pool, tc.tile_pool(
                name="enc_w", bufs=2
            ) as w_pool, tc.tile_pool(name="enc_benc", bufs=1) as benc_pool, tc.tile_pool(
                name="enc_acts", bufs=BT_Q
            ) as acts_pool, tc.tile_pool(
                name="enc_small", bufs=4
            ) as small_pool, tc.tile_pool(
                name="enc_psum", bufs=2 * BT_Q, space="PSUM"
            ) as mm_psum:
                xT_r = {
                    "h": xTh.rearrange("(t p) b -> p t b", p=P),
                    "l": xTl.rearrange("(t p) b -> p t b", p=P),
                }
                for q in range(4):
                    qs = slice(q * QB, (q + 1) * QB)
                    xh = x_pool.tile([P, NDT, QB], dt.float32r, tag="xh", name="xh")
                    nc.sync.dma_start(xh[:], xT_r["h"][:, :, qs])
                    xl = x_pool.tile([P, NDT, QB], dt.float32r, tag="xl", name="xl")
                    nc.sync.dma_start(xl[:], xT_r["l"][:, :, qs])
                    for k in range(KL):
                        benc_rep = benc_pool.tile([P, G], dt.float32, tag="benc", name="benc")
                        nc.sync.dma_start(
                            benc_rep[:], benc_eff[k : k + 1, :].to_broadcast((P, G))
                        )
                        acts_tiles = [
                            acts_pool.tile([P, G], dt.float32, tag="acts", name="acts")
                            for _ in range(BT_Q)
                        ]
                        for c in range(NCH):
                            cs = slice(c * CH, (c + 1) * CH)
                            eh = w_pool.tile([P, NDT, CH], dt.float32r, tag="w", name="eh")
                            nc.sync.dma_start(
                                eh[:], encTh[k].rearrange("(t p) g -> p t g", p=P)[:, :, cs]
                            )
                            el = w_pool.tile([P, NDT, CH], dt.float32r, tag="w", name="el")
                            nc.sync.dma_start(
                                el[:], encTl[k].rearrange("(t p) g -> p t g", p=P)[:, :, cs]
                            )
                            pss = [
                                mm_psum.tile([P, CH], dt.float32, tag="mm", name="mm")
                                for _ in range(BT_Q)
                            ]
                            for xt, et, first, last in (
                                (xh, eh, True, False),
                                (xl, eh, False, False),
                                (xh, el, False, True),
                            ):
                                for bt in range(BT_Q):
                                    for t in range(NDT):
                                        nc.tensor.matmul(
                                            pss[bt][:],
                                            xt[:, t, bt * P : (bt + 1) * P],
                                            et[:, t, :],
                                            start=(first and t == 0),
                                            stop=(last and t == NDT - 1),
                                        )
                            for bt in range(BT_Q):
                                nc.vector.tensor_tensor(
                                    out=acts_tiles[bt][:, cs],
                                    in0=pss[bt][:],
                                    in1=benc_rep[:, cs],
                                    op=Alu.add,
                                )
                        for bt in range(BT_Q):
                            gbt = q * BT_Q + bt
                            mx8 = small_pool.tile([P, 8], dt.float32, tag="mx8", name="mx8")
                            nc.vector.max(mx8[:], acts_tiles[bt][:])
                            mi8 = small_pool.tile([P, 8], dt.uint32, tag="mi8", name="mi8")
                            nc.vector.max_index(mi8[:], mx8[:], acts_tiles[bt][:])
                            nc.vector.tensor_copy(maxv_all[:, gbt, k : k + 1], mx8[:, 0:1])
                            nc.vector.tensor_copy(idx_all[:, gbt, k : k + 1], mi8[:, 0:1])

            nc.sync.dma_start(idx_dbg.rearrange("(t p) k -> p t k", p=P), idx_all[:])
            nc.sync.dma_start(maxv_dbg.rearrange("(t p) k -> p t k", p=P), maxv_all[:])

            # ================= counts + decode (overlap) =================
            with tc.tile_pool(name="pd_gath", bufs=3) as gath_pool, tc.tile_pool(
                name="pd_acc", bufs=2
            ) as acc_pool, tc.tile_pool(name="pd_small", bufs=6) as psmall, tc.tile_pool(
                name="pd_cnt_psum", bufs=1, space="PSUM"
            ) as cnt_psum, tc.tile_pool(name="pd_cnt_sb", bufs=4) as cnt_sb_pool:
                # ---- feature counts per local group
                for k in range(KL):
                    cps = [
                        cnt_psum.tile([1, 512], dt.float32, tag=f"cnt{c}", name=f"cnt{c}")
                        for c in range(4)
                    ]
                    for bt in range(NBT):
                        idx_f = psmall.tile([P, 1], dt.float32, tag="idxf", name="idxf")
                        nc.vector.tensor_copy(idx_f[:], idx_all[:, bt, k : k + 1])
                        mask = psmall.tile([P, G], dt.bfloat16, tag="mask", name="mask")
                        nc.vector.tensor_scalar(
                            out=mask[:], in0=iota_f[:], scalar1=idx_f[:],
                            scalar2=None, op0=Alu.is_equal,
                        )
                        for c in range(4):
                            nc.tensor.matmul(
                                cps[c][:], ones_bf[:], mask[:, c * 512 : (c + 1) * 512],
                                start=(bt == 0), stop=(bt == NBT - 1),
                            )
                    for c in range(4):
                        cnt_i = cnt_sb_pool.tile([1, 512], dt.int32, tag="cnti", name="cnti")
                        nc.vector.tensor_copy(cnt_i[:], cps[c][:])
                        nc.sync.dma_start(
                            counts_part[k : k + 1, c * 512 : (c + 1) * 512], cnt_i[:]
                        )

                # ---- decode: out_part rows = sum_k maxv * dec[k*G + idx]
                def decode_tile(bt, dest):
                    acc = acc_pool.tile([P, D], dt.float32, tag="acc", name="acc")
                    for k in range(KL):
                        idxg = psmall.tile([P, 1], dt.uint32, tag="idxg", name="idxg")
                        nc.vector.tensor_scalar(
                            out=idxg[:], in0=idx_all[:, bt, k : k + 1],
                            scalar1=k * G, scalar2=None, op0=Alu.add,
                        )
                        gath = gath_pool.tile([P, D], dt.float32, tag="gath", name="gath")
                        nc.gpsimd.indirect_dma_start(
                            out=gath[:], out_offset=None, in_=dec,
                            in_offset=bass.IndirectOffsetOnAxis(ap=idxg[:], axis=0),
                        )
                        mv = maxv_all[:, bt, k : k + 1]
                        if k == 0:
                            nc.vector.tensor_scalar(
                                out=acc[:], in0=gath[:], scalar1=mv,
                                scalar2=None, op0=Alu.mult,
                            )
                        else:
                            nc.vector.scalar_tensor_tensor(
                                out=acc[:], in0=gath[:], scalar=mv, in1=acc[:],
                                op0=Alu.mult, op1=Alu.add,
                            )
                    nc.sync.dma_start(dest, acc[:])

                for c2 in range(NCORE):  # even tiles -> out_partA
                    decode_tile(2 * c2, out_partA[c2 * P : (c2 + 1) * P, :])
                nc.gpsimd.collective_compute(
                    "ReduceScatter", Alu.add,
                    replica_groups=[list(range(NCORE))],
                    ins=[out_partA], outs=[rs_outA],
                )
                for c2 in range(NCORE):  # odd tiles -> out_partB (overlaps RS-A)
                    decode_tile(2 * c2 + 1, out_partB[c2 * P : (c2 + 1) * P, :])
                nc.gpsimd.collective_compute(
                    "ReduceScatter", Alu.add,
                    replica_groups=[list(range(NCORE))],
                    ins=[out_partB], outs=[rs_outB],
                )

            # ================= fvu partials on own 256-token slice =================
            with tc.tile_pool(name="fv", bufs=2) as fv_pool, tc.tile_pool(
                name="fv_acc", bufs=1
            ) as fvacc_pool, tc.tile_pool(name="fv_psum", bufs=1, space="PSUM") as fv_psum:
                bdrep = fvacc_pool.tile([P, D], dt.float32)
                nc.sync.dma_start(bdrep[:], b_dec_row.to_broadcast((P, D)))
                e_acc = fvacc_pool.tile([P, 1], dt.float32)
                nc.any.memset(e_acc[:], 0.0)
                sq_acc = fvacc_pool.tile([P, 1], dt.float32)
                nc.any.memset(sq_acc[:], 0.0)
                csps = [
                    fv_psum.tile([1, 512], dt.float32, tag=f"cs{c}", name=f"cs{c}")
                    for c in range(4)
                ]
                nbt_s = BS // P  # 2
                for bt in range(nbt_s):
                    rs_src = rs_outA if bt == 0 else rs_outB
                    xc = fv_pool.tile([P, D], dt.float32, tag="xc", name="xc")
                    nc.sync.dma_start(xc[:], xs[bt * P : (bt + 1) * P, :])
                    nc.vector.tensor_tensor(out=xc[:], in0=xc[:], in1=bdrep[:], op=Alu.subtract)
                    ot = fv_pool.tile([P, D], dt.float32, tag="ot", name="ot")
                    nc.sync.dma_start(ot[:], rs_src)
                    # diff = xc - (ot + b_dec)
                    nc.vector.tensor_tensor(out=ot[:], in0=ot[:], in1=bdrep[:], op=Alu.add)
                    nc.vector.tensor_tensor(out=ot[:], in0=xc[:], in1=ot[:], op=Alu.subtract)
                    nc.vector.tensor_tensor(out=ot[:], in0=ot[:], in1=ot[:], op=Alu.mult)
                    red = fv_pool.tile([P, 1], dt.float32, tag="red", name="red")
                    nc.vector.tensor_reduce(red[:], ot[:], axis=mybir.AxisListType.X, op=Alu.add)
                    nc.vector.tensor_tensor(out=e_acc[:], in0=e_acc[:], in1=red[:], op=Alu.add)
                    for c in range(4):
                        nc.tensor.matmul(
                            csps[c][:], ones_f[:], xc[:, c * 512 : (c + 1) * 512],
                            start=(bt == 0), stop=(bt == nbt_s - 1),
                        )
                    nc.vector.tensor_tensor(out=xc[:], in0=xc[:], in1=xc[:], op=Alu.mult)
                    red2 = fv_pool.tile([P, 1], dt.float32, tag="red2", name="red2")
                    nc.vector.tensor_reduce(red2[:], xc[:], axis=mybir.AxisListType.X, op=Alu.add)
                    nc.vector.tensor_tensor(out=sq_acc[:], in0=sq_acc[:], in1=red2[:], op=Alu.add)
                for c in range(4):
                    cs_sb = fv_pool.tile([1, 512], dt.float32, tag="cs_sb", name="cs_sb")
                    nc.vector.tensor_copy(cs_sb[:], csps[c][:])
                    nc.sync.dma_start(colsum_part[:, c * 512 : (c + 1) * 512], cs_sb[:])
                e_s = fv_pool.tile([1, 1], dt.float32, tag="es", name="es")
                nc.gpsimd.tensor_reduce(e_s[:], e_acc[:], axis=mybir.AxisListType.XYZWC, op=Alu.add)
                nc.sync.dma_start(e_part, e_s[:])
                sq_s = fv_pool.tile([1, 1], dt.float32, tag="sqs", name="sqs")
                nc.gpsimd.tensor_reduce(sq_s[:], sq_acc[:], axis=mybir.AxisListType.XYZWC, op=Alu.add)
                nc.sync.dma_start(sq_part, sq_s[:])

    nc.compile()
    return nc


def _get_nc():
    global _nc_cache
    if _nc_cache is None:
        _nc_cache = _build()
    return _nc_cache


def kernel(x, encoder, b_enc, decoder, b_dec, _trace=False):
    x = np.ascontiguousarray(x, dtype=np.float32)
    encoder = np.asarray(encoder, dtype=np.float32)
    decoder = np.asarray(decoder, dtype=np.float32)
    b_enc = np.ascontiguousarray(b_enc, dtype=np.float32)
    b_dec = np.ascontiguousarray(b_dec, dtype=np.float32)

    xT = np.ascontiguousarray(x.T)
    xTh = _f32r_round(xT)
    xTl = _f32r_round(xT - xTh)
    b_dec_row = np.ascontiguousarray(b_dec.reshape(1, D))
    # benc_eff[k,g] = b_enc[k,g] - sum_d encoder[k,g,d]*b_dec[d], in fp64
    benc_eff_all = (
        b_enc.astype(np.float64)
        - np.einsum("kgd,d->kg", encoder.astype(np.float64), b_dec.astype(np.float64))
    ).astype(np.float32)

    in_maps = []
    for c in range(NCORE):
        ks = slice(c * KL, (c + 1) * KL)
        encT = np.ascontiguousarray(encoder[ks].transpose(0, 2, 1))
        encTh = _f32r_round(encT)
        encTl = _f32r_round(encT - encTh)
        in_maps.append(
            dict(
                xTh=xTh,
                xTl=xTl,
                xs=np.ascontiguousarray(x[c * BS : (c + 1) * BS]),
                b_dec_row=b_dec_row,
                encTh=encTh,
                encTl=encTl,
                benc_eff=np.ascontiguousarray(benc_eff_all[ks]),
                dec=np.ascontiguousarray(decoder[ks].reshape(KL * G, D)),
            )
        )

    nc = _get_nc()
    res = run_bass_kernel_spmd(nc, in_maps, list(range(NCORE)), trace=_trace)
    results = res.results

    counts = np.concatenate(
        [results[c]["counts_part"].reshape(-1) for c in range(NCORE)]
    ).astype(np.int32)
    e = np.float64(sum(float(results[c]["e_part"][0, 0]) for c in range(NCORE)))
    sq = np.float64(sum(float(results[c]["sq_part"][0, 0]) for c in range(NCORE)))
    colsum = np.sum(
        [results[c]["colsum_part"][0].astype(np.float64) for c in range(NCORE)], axis=0
    )
    tv = sq - float(np.sum(colsum * colsum)) / B
    fvu = np.float32(e / tv)
    if _trace:
        kernel._last_perf = res
    return fvu, counts, np.float32(0.0)


if __name__ == "__main__":
    _get_nc()
    print("build+compile OK")


# revision 7
# speedup vs baseline: 1.3290x; 1.0936x over previous
"""Grouped top-1 sparse autoencoder (vq_codebook) on 8 Trainium2 NeuronCores.

Sharding: expert-style over the K=32 group axis (4 groups per core) for the
encode GEMM + top-1 + gather-decode; per-core partial reconstructions are
ReduceScatter-summed on device (split in two halves to overlap with decode)
so each core finishes the FVU partial sums for its 256-token slice.

Encode GEMM runs in float32r (TRN2 tfloat32: 12-bit significand) with an
exact hi/lo split of both operands done on the host; three f32r passes
(hi*hi + hi*lo + lo*hi) reproduce fp32 accuracy at ~1 cycle/row instead of
fp32's 4. b_dec is folded into an effective encoder bias computed in fp64 on
the host: (x-b_dec)@enc^T + b_enc == x@enc^T + (b_enc - enc@b_dec).

Host work: input layout prep (transpose/slice/split) and summing the tiny
per-core partials (e, sum(xc^2), column sums, feature counts).
"""

import sys

for _p in ("/opt/trn_rl_repo",):
    if _p not in sys.path:
        sys.path.insert(0, _p)

import numpy as np
import concourse.bass as bass
import concourse.bacc as bacc
import concourse.tile as tile
from concourse import mybir
from concourse.bass_utils import run_bass_kernel_spmd

dt = mybir.dt
Alu = mybir.AluOpType

P = 128
K = 32  # groups
G = 2048  # latents per group
D = 2048  # d_model
B = 2048  # batch
NCORE = 8
KL = K // NCORE  # 4 groups per core
BS = B // NCORE  # 256 tokens per core for the fvu phase
NDT = D // P  # 16 contraction tiles
CH = 512  # encode matmul moving chunk (N)
NCH = G // CH  # 4 chunks per group
QB = B // 4  # 512 tokens per resident x block
BT_Q = QB // P  # 4 B-tiles per block
NBT = B // P  # 16 B-tiles total

_nc_cache = None


def _f32r_round(x: np.ndarray) -> np.ndarray:
    """Round fp32 to nearest 12-bit-significand value (matches PE f32r RNE)."""
    u = x.astype(np.float32).view(np.uint32).astype(np.uint64)
    drop = 12
    bias = ((u >> np.uint64(drop)) & np.uint64(1)) + (
        (np.uint64(1) << np.uint64(drop - 1)) - np.uint64(1)
    )
    u2 = ((u + bias) & ~np.uint64((1 << drop) - 1)).astype(np.uint32)
    return u2.view(np.float32)


def _build():
    nc = bacc.Bacc("TRN2", target_bir_lowering=False, debug=False, num_devices=NCORE)

    xTh = nc.dram_tensor("xTh", [D, B], dt.float32r, kind="ExternalInput").ap()
    xTl = nc.dram_tensor("xTl", [D, B], dt.float32r, kind="ExternalInput").ap()
    xs = nc.dram_tensor("xs", [BS, D], dt.float32, kind="ExternalInput").ap()
    b_dec_row = nc.dram_tensor("b_dec_row", [1, D], dt.float32, kind="ExternalInput").ap()
    encTh = nc.dram_tensor("encTh", [KL, D, G], dt.float32r, kind="ExternalInput").ap()
    encTl = nc.dram_tensor("encTl", [KL, D, G], dt.float32r, kind="ExternalInput").ap()
    benc_eff = nc.dram_tensor("benc_eff", [KL, G], dt.float32, kind="ExternalInput").ap()
    dec = nc.dram_tensor("dec", [KL * G, D], dt.float32, kind="ExternalInput").ap()

    counts_part = nc.dram_tensor("counts_part", [KL, G], dt.int32, kind="ExternalOutput").ap()
    e_part = nc.dram_tensor("e_part", [1, 1], dt.float32, kind="ExternalOutput").ap()
    sq_part = nc.dram_tensor("sq_part", [1, 1], dt.float32, kind="ExternalOutput").ap()
    colsum_part = nc.dram_tensor("colsum_part", [1, D], dt.float32, kind="ExternalOutput").ap()
    idx_dbg = nc.dram_tensor("idx_dbg", [B, KL], dt.uint32, kind="ExternalOutput").ap()
    maxv_dbg = nc.dram_tensor("maxv_dbg", [B, KL], dt.float32, kind="ExternalOutput").ap()

    # partial reconstructions, split so RS(A) (tokens 0..1023, b-tiles 0..7,
    # decoded after encode quarters 0-1) overlaps encode of quarters 2-3.
    # RS gives core c rows [128c, 128c+128) of each half; the host slices xs
    # to the matching token sets.
    out_partA = nc.dram_tensor("out_partA", [B // 2, D], dt.float32).ap()
    out_partB = nc.dram_tensor("out_partB", [B // 2, D], dt.float32).ap()
    rs_outA = nc.dram_tensor("rs_outA", [P, D], dt.float32).ap()
    rs_outB = nc.dram_tensor("rs_outB", [P, D], dt.float32).ap()

    with tile.TileContext(nc) as tc:
        with tc.tile_pool(name="const", bufs=1) as cpool:
            iota_f = cpool.tile([P, G], dt.float32)
            with tc.tile_pool(name="iota_tmp", bufs=1) as itmp:
                iota_i = itmp.tile([P, G], dt.int32)
                nc.gpsimd.iota(iota_i[:], pattern=[[1, G]], base=0, channel_multiplier=0)
                nc.vector.tensor_copy(iota_f[:], iota_i[:])
            ones_bf = cpool.tile([P, 1], dt.bfloat16)
            nc.any.memset(ones_bf[:], 1.0)
            ones_f = cpool.tile([P, 1], dt.float32)
            nc.any.memset(ones_f[:], 1.0)
            idx_all = cpool.tile([P, NBT, KL], dt.uint32)
            maxv_all = cpool.tile([P, NBT, KL], dt.float32)

            # ================= encode =================
            with tc.tile_pool(name="enc_x", bufs=1) as x_pool, tc.tile_pool(
                name="enc_w", bufs=2
            ) as w_pool, tc.tile_pool(name="enc_benc", bufs=1) as benc_pool, tc.tile_pool(
                name="enc_acts", bufs=BT_Q
            ) as acts_pool, tc.tile_pool(
                name="enc_small", bufs=4
            ) as small_pool, tc.tile_pool(
                name="enc_psum", bufs=2 * BT_Q, space="PSUM"
            ) as mm_psum, tc.tile_pool(
                name="dec_gath", bufs=2
            ) as gath_pool, tc.tile_pool(name="dec_acc", bufs=1) as acc_pool:

                def decode_tile(bt):
                    # out_part row layout: half A = tokens [0,1024), B = rest
                    dest = (
                        out_partA[bt * P : (bt + 1) * P, :]
                        if bt < NBT // 2
                        else out_partB[(bt - NBT // 2) * P : (bt - NBT // 2 + 1) * P, :]
                    )
                    acc = acc_pool.tile([P, D], dt.float32, tag="acc", name="acc")
                    for k in range(KL):
                        idxg = small_pool.tile([P, 1], dt.uint32, tag="idxg", name="idxg")
                        nc.vector.tensor_scalar(
                            out=idxg[:], in0=idx_all[:, bt, k : k + 1],
                            scalar1=k * G, scalar2=None, op0=Alu.add,
                        )
                        gath = gath_pool.tile([P, D], dt.float32, tag="gath", name="gath")
                        nc.gpsimd.indirect_dma_start(
                            out=gath[:], out_offset=None, in_=dec,
                            in_offset=bass.IndirectOffsetOnAxis(ap=idxg[:], axis=0),
                        )
                        mv = maxv_all[:, bt, k : k + 1]
                        if k == 0:
                            nc.vector.tensor_scalar(
                                out=acc[:], in0=gath[:], scalar1=mv,
                                scalar2=None, op0=Alu.mult,
                            )
                        else:
                            nc.vector.scalar_tensor_tensor(
                                out=acc[:], in0=gath[:], scalar=mv, in1=acc[:],
                                op0=Alu.mult, op1=Alu.add,
                            )
                    nc.sync.dma_start(dest, acc[:])

                xT_r = {
                    "h": xTh.rearrange("(t p) b -> p t b", p=P),
                    "l": xTl.rearrange("(t p) b -> p t b", p=P),
                }
                for q in range(4):
                    qs = slice(q * QB, (q + 1) * QB)
                    xh = x_pool.tile([P, NDT, QB], dt.float32r, tag="xh", name="xh")
                    xl = x_pool.tile([P, NDT, QB], dt.float32r, tag="xl", name="xl")
                    for t in range(NDT):
                        nc.sync.dma_start(xh[:, t, :], xT_r["h"][:, t, qs])
                        nc.sync.dma_start(xl[:, t, :], xT_r["l"][:, t, qs])
                    for k in range(KL):
                        benc_rep = benc_pool.tile([P, G], dt.float32, tag="benc", name="benc")
                        nc.sync.dma_start(
                            benc_rep[:], benc_eff[k : k + 1, :].to_broadcast((P, G))
                        )
                        acts_tiles = [
                            acts_pool.tile([P, G], dt.float32, tag="acts", name="acts")
                            for _ in range(BT_Q)
                        ]
                        for c in range(NCH):
                            cs = slice(c * CH, (c + 1) * CH)
                            eh = w_pool.tile([P, NDT, CH], dt.float32r, tag="w", name="eh")
                            nc.sync.dma_start(
                                eh[:], encTh[k].rearrange("(t p) g -> p t g", p=P)[:, :, cs]
                            )
                            el = w_pool.tile([P, NDT, CH], dt.float32r, tag="w", name="el")
                            nc.sync.dma_start(
                                el[:], encTl[k].rearrange("(t p) g -> p t g", p=P)[:, :, cs]
                            )
                            pss = [
                                mm_psum.tile([P, CH], dt.float32, tag="mm", name="mm")
                                for _ in range(BT_Q)
                            ]
                            for xt, et, first, last in (
                                (xh, eh, True, False),
                                (xl, eh, False, False),
                                (xh, el, False, True),
                            ):
                                for bt in range(BT_Q):
                                    for t in range(NDT):
                                        nc.tensor.matmul(
                                            pss[bt][:],
                                            xt[:, t, bt * P : (bt + 1) * P],
                                            et[:, t, :],
                                            start=(first and t == 0),
                                            stop=(last and t == NDT - 1),
                                        )
                            for bt in range(BT_Q):
                                nc.vector.tensor_tensor(
                                    out=acts_tiles[bt][:, cs],
                                    in0=pss[bt][:],
                                    in1=benc_rep[:, cs],
                                    op=Alu.add,
                                )
                        for bt in range(BT_Q):
                            gbt = q * BT_Q + bt
                            mx8 = small_pool.tile([P, 8], dt.float32, tag="mx8", name="mx8")
                            nc.vector.max(mx8[:], acts_tiles[bt][:])
                            mi8 = small_pool.tile([P, 8], dt.uint32, tag="mi8", name="mi8")
                            nc.vector.max_index(mi8[:], mx8[:], acts_tiles[bt][:])
                            nc.vector.tensor_copy(maxv_all[:, gbt, k : k + 1], mx8[:, 0:1])
                            nc.vector.tensor_copy(idx_all[:, gbt, k : k + 1], mi8[:, 0:1])
                    for bt in range(BT_Q):
                        decode_tile(q * BT_Q + bt)
                    if q == 1:
                        nc.gpsimd.collective_compute(
                            "ReduceScatter", Alu.add,
                            replica_groups=[list(range(NCORE))],
                            ins=[out_partA], outs=[rs_outA],
                        )

            nc.gpsimd.collective_compute(
                "ReduceScatter", Alu.add,
                replica_groups=[list(range(NCORE))],
                ins=[out_partB], outs=[rs_outB],
            )
            nc.sync.dma_start(idx_dbg.rearrange("(t p) k -> p t k", p=P), idx_all[:])
            nc.sync.dma_start(maxv_dbg.rearrange("(t p) k -> p t k", p=P), maxv_all[:])

            # ================= counts + decode (overlap) =================
            with tc.tile_pool(name="pd_small", bufs=6) as psmall, tc.tile_pool(
                name="pd_cnt_psum", bufs=1, space="PSUM"
            ) as cnt_psum, tc.tile_pool(name="pd_cnt_sb", bufs=4) as cnt_sb_pool:
                # ---- feature counts per local group
                for k in range(KL):
                    cps = [
                        cnt_psum.tile([1, 512], dt.float32, tag=f"cnt{c}", name=f"cnt{c}")
                        for c in range(4)
                    ]
                    for bt in range(NBT):
                        idx_f = psmall.tile([P, 1], dt.float32, tag="idxf", name="idxf")
                        nc.vector.tensor_copy(idx_f[:], idx_all[:, bt, k : k + 1])
                        mask = psmall.tile([P, G], dt.bfloat16, tag="mask", name="mask")
                        nc.vector.tensor_scalar(
                            out=mask[:], in0=iota_f[:], scalar1=idx_f[:],
                            scalar2=None, op0=Alu.is_equal,
                        )
                        for c in range(4):
                            nc.tensor.matmul(
                                cps[c][:], ones_bf[:], mask[:, c * 512 : (c + 1) * 512],
                                start=(bt == 0), stop=(bt == NBT - 1),
                            )
                    for c in range(4):
                        cnt_i = cnt_sb_pool.tile([1, 512], dt.int32, tag="cnti", name="cnti")
                        nc.vector.tensor_copy(cnt_i[:], cps[c][:])
                        nc.sync.dma_start(
                            counts_part[k : k + 1, c * 512 : (c + 1) * 512], cnt_i[:]
                        )


            # ================= fvu partials on own 256-token slice =================
            with tc.tile_pool(name="fv", bufs=2) as fv_pool, tc.tile_pool(
                name="fv_acc", bufs=1
            ) as fvacc_pool, tc.tile_pool(name="fv_psum", bufs=1, space="PSUM") as fv_psum:
                bdrep = fvacc_pool.tile([P, D], dt.float32)
                nc.sync.dma_start(bdrep[:], b_dec_row.to_broadcast((P, D)))
                e_acc = fvacc_pool.tile([P, 1], dt.float32)
                nc.any.memset(e_acc[:], 0.0)
                sq_acc = fvacc_pool.tile([P, 1], dt.float32)
                nc.any.memset(sq_acc[:], 0.0)
                csps = [
                    fv_psum.tile([1, 512], dt.float32, tag=f"cs{c}", name=f"cs{c}")
                    for c in range(4)
                ]
                nbt_s = BS // P  # 2
                for bt in range(nbt_s):
                    rs_src = rs_outA if bt == 0 else rs_outB
                    xc = fv_pool.tile([P, D], dt.float32, tag="xc", name="xc")
                    nc.sync.dma_start(xc[:], xs[bt * P : (bt + 1) * P, :])
                    nc.vector.tensor_tensor(out=xc[:], in0=xc[:], in1=bdrep[:], op=Alu.subtract)
                    ot = fv_pool.tile([P, D], dt.float32, tag="ot", name="ot")
                    nc.sync.dma_start(ot[:], rs_src)
                    # diff = xc - (ot + b_dec)
                    nc.vector.tensor_tensor(out=ot[:], in0=ot[:], in1=bdrep[:], op=Alu.add)
                    nc.vector.tensor_tensor(out=ot[:], in0=xc[:], in1=ot[:], op=Alu.subtract)
                    nc.vector.tensor_tensor(out=ot[:], in0=ot[:], in1=ot[:], op=Alu.mult)
                    red = fv_pool.tile([P, 1], dt.float32, tag="red", name="red")
                    nc.vector.tensor_reduce(red[:], ot[:], axis=mybir.AxisListType.X, op=Alu.add)
                    nc.vector.tensor_tensor(out=e_acc[:], in0=e_acc[:], in1=red[:], op=Alu.add)
                    for c in range(4):
                        nc.tensor.matmul(
                            csps[c][:], ones_f[:], xc[:, c * 512 : (c + 1) * 512],
                            start=(bt == 0), stop=(bt == nbt_s - 1),
                        )
                    nc.vector.tensor_tensor(out=xc[:], in0=xc[:], in1=xc[:], op=Alu.mult)
                    red2 = fv_pool.tile([P, 1], dt.float32, tag="red2", name="red2")
                    nc.vector.tensor_reduce(red2[:], xc[:], axis=mybir.AxisListType.X, op=Alu.add)
                    nc.vector.tensor_tensor(out=sq_acc[:], in0=sq_acc[:], in1=red2[:], op=Alu.add)
                for c in range(4):
                    cs_sb = fv_pool.tile([1, 512], dt.float32, tag="cs_sb", name="cs_sb")
                    nc.vector.tensor_copy(cs_sb[:], csps[c][:])
                    nc.sync.dma_start(colsum_part[:, c * 512 : (c + 1) * 512], cs_sb[:])
                e_s = fv_pool.tile([1, 1], dt.float32, tag="es", name="es")
                nc.gpsimd.tensor_reduce(e_s[:], e_acc[:], axis=mybir.AxisListType.XYZWC, op=Alu.add)
                nc.sync.dma_start(e_part, e_s[:])
                sq_s = fv_pool.tile([1, 1], dt.float32, tag="sqs", name="sqs")
                nc.gpsimd.tensor_reduce(sq_s[:], sq_acc[:], axis=mybir.AxisListType.XYZWC, op=Alu.add)
                nc.sync.dma_start(sq_part, sq_s[:])

    nc.compile()
    return nc


def _get_nc():
    global _nc_cache
    if _nc_cache is None:
        _nc_cache = _build()
    return _nc_cache


def kernel(x, encoder, b_enc, decoder, b_dec, _trace=False):
    x = np.ascontiguousarray(x, dtype=np.float32)
    encoder = np.asarray(encoder, dtype=np.float32)
    decoder = np.asarray(decoder, dtype=np.float32)
    b_enc = np.ascontiguousarray(b_enc, dtype=np.float32)
    b_dec = np.ascontiguousarray(b_dec, dtype=np.float32)

    xT = np.ascontiguousarray(x.T)
    xTh = _f32r_round(xT)
    xTl = _f32r_round(xT - xTh)
    b_dec_row = np.ascontiguousarray(b_dec.reshape(1, D))
    # benc_eff[k,g] = b_enc[k,g] - sum_d encoder[k,g,d]*b_dec[d], in fp64
    benc_eff_all = (
        b_enc.astype(np.float64)
        - np.einsum("kgd,d->kg", encoder.astype(np.float64), b_dec.astype(np.float64))
    ).astype(np.float32)

    in_maps = []
    for c in range(NCORE):
        ks = slice(c * KL, (c + 1) * KL)
        encT = np.ascontiguousarray(encoder[ks].transpose(0, 2, 1))
        encTh = _f32r_round(encT)
        encTl = _f32r_round(encT - encTh)
        in_maps.append(
            dict(
                xTh=xTh,
                xTl=xTl,
                xs=np.ascontiguousarray(
                    np.concatenate(
                        [x[c * P : (c + 1) * P], x[B // 2 + c * P : B // 2 + (c + 1) * P]]
                    )
                ),
                b_dec_row=b_dec_row,
                encTh=encTh,
                encTl=encTl,
                benc_eff=np.ascontiguousarray(benc_eff_all[ks]),
                dec=np.ascontiguousarray(decoder[ks].reshape(KL * G, D)),
            )
        )

    nc = _get_nc()
    res = run_bass_kernel_spmd(nc, in_maps, list(range(NCORE)), trace=_trace)
    results = res.results

    counts = np.concatenate(
        [results[c]["counts_part"].reshape(-1) for c in range(NCORE)]
    ).astype(np.int32)
    e = np.float64(sum(float(results[c]["e_part"][0, 0]) for c in range(NCORE)))
    sq = np.float64(sum(float(results[c]["sq_part"][0, 0]) for c in range(NCORE)))
    colsum = np.sum(
        [results[c]["colsum_part"][0].astype(np.float64) for c in range(NCORE)], axis=0
    )
    tv = sq - float(np.sum(colsum * colsum)) / B
    fvu = np.float32(e / tv)
    if _trace:
        kernel._last_perf = res
    return fvu, counts, np.float32(0.0)


if __name__ == "__main__":
    _get_nc()
    print("build+compile OK")
